# revision 14
# baseline (speedup 1.0000x reference)
"""Block-sparse (view-causal) multi-head attention on 8 TRN2 NeuronCores.

Full inputs in, full output out. Sharding: data-parallel over batch (B=2),
tensor-parallel over heads (16 heads -> 4 per core). Each core computes its
4 heads' attention + its slice of the output projection; the host sums the
4 head-group partial projections per batch (the tensor-parallel reduce).

v5: micro-unit software pipelining. The emission stream alternates single
score blocks (4 matmuls + exp) with ~0.5us filler units (v-projection
chunks, the second head-pair's q/k projection, PV groups, transposes,
output-projection chunks) pulled from a FIFO, so the PE never stalls on
the 2-deep score-PSUM ring waiting for the ACT exp drain, and the ACT exp
stream starts as soon as the first score block exists (~15us). Input DMA
is batched into a few large strided transfers, first token-quarter first.

Device-side layout: activations kept transposed (qT/kT [dh, tokens]) so the
score matmuls need no transposes and the two K=64 head matmuls row-pack in
the PE array; V is augmented with a ones column so each PV matmul
accumulates both sum(exp*v) and the softmax denominator in PSUM. All
matmul operands are bf16 (PSUM accumulation in fp32).
"""

import sys

if "/opt/trn_rl_repo" not in sys.path:
    sys.path.insert(0, "/opt/trn_rl_repo")

import numpy as np
import ml_dtypes
from collections import deque

B, V, L, C, H = 2, 8, 256, 1024, 16
S = V * L                # 2048 tokens
DH = C // H              # 64
HPC = 4                  # heads per core
CPB = HPC * DH           # 256 channel block per core
N_CORES = 8
SCALE = DH ** -0.5       # 1/8, folded into the exp activation

_compiled = {}
LAST_RESULTS = None
PACK_QK = True           # kept for test.py compat
SAFE_RECIP = False
SPLIT_ACT = False

KC = C // 128            # 8 contraction chunks for the projections
SC = S // 128            # 16 sequence chunks


def _allowed(qv):
    """View-level mask row: views 0/1 cross-attend only; views >=2 block-causal."""
    if qv == 0:
        return [1]
    if qv == 1:
        return [0]
    return list(range(qv + 1))


def build():
    import concourse.tile as tile
    from concourse import bacc, mybir
    from concourse.masks import make_identity

    f32 = mybir.dt.float32
    bf16 = mybir.dt.bfloat16
    EXP = mybir.ActivationFunctionType.Exp

    nc = bacc.Bacc("TRN2", target_bir_lowering=False, debug=False,
                   num_devices=N_CORES)
    xT = nc.dram_tensor("xT", [C, S], bf16, kind="ExternalInput").ap()
    wqT = nc.dram_tensor("wqT", [C, CPB], bf16, kind="ExternalInput").ap()
    wkT = nc.dram_tensor("wkT", [C, CPB], bf16, kind="ExternalInput").ap()
    wvT = nc.dram_tensor("wvT", [C, CPB], bf16, kind="ExternalInput").ap()
    wpT = nc.dram_tensor("wpT", [CPB, C], bf16, kind="ExternalInput").ap()
    y = nc.dram_tensor("y", [S, C], bf16, kind="ExternalOutput").ap()

    with tile.TileContext(nc) as tc:
        with (
            tc.tile_pool(name="xt", bufs=1) as xt_pool,
            tc.tile_pool(name="wts", bufs=1) as w_pool,
            tc.tile_pool(name="qk", bufs=1) as qk_pool,
            tc.tile_pool(name="va", bufs=SC) as va_pool,
            tc.tile_pool(name="ot", bufs=1) as ot_pool,
            tc.tile_pool(name="exp", bufs=22) as exp_pool,
            tc.tile_pool(name="small", bufs=1) as small_pool,
            tc.tile_pool(name="ysb", bufs=4) as ysb_pool,
            tc.tile_pool(name="psb", bufs=2, space="PSUM") as ps_big,
            tc.tile_pool(name="pss", bufs=4, space="PSUM") as ps_sm,
        ):
            # ---- input DMAs: few large strided transfers ----
            wq_m, wk_m = [], []
            for m in range(2):
                tq = w_pool.tile([128, KC * 128], bf16, tag=f"wq{m}",
                                 name=f"wq{m}")
                tk = w_pool.tile([128, KC * 128], bf16, tag=f"wk{m}",
                                 name=f"wk{m}")
                wq_m.append(tq)
                wk_m.append(tk)
            wv_b = w_pool.tile([128, KC * CPB], bf16, tag="wv", name="wv")
            wp_b = w_pool.tile([128, 2 * C], bf16, tag="wp", name="wp")

            def w_in(dram_ap, cols):
                return dram_ap[:, cols].rearrange("(k p) c -> p k c", p=128)

            xbig = xt_pool.tile([128, KC * S], bf16, tag="xt", name="xbig")

            def xts(k):
                return xbig[:, k * S:(k + 1) * S]

            # strict priority: the m0 q/k weights and the first token-quarter
            # of x (in two chunk-halves) gate the first score block — they go
            # first on the sync ring; everything else follows on scalar.
            nc.sync.dma_start(
                wq_m[0][:].rearrange("p (k c) -> p k c", c=128),
                w_in(wqT, slice(0, 128)))
            nc.sync.dma_start(
                wk_m[0][:].rearrange("p (k c) -> p k c", c=128),
                w_in(wkT, slice(0, 128)))
            for ks in (slice(0, 4), slice(4, 8)):
                nc.sync.dma_start(
                    xbig[:].rearrange("p (k s) -> p k s", s=S)[:, ks, 0:512],
                    xT[:, 0:512].rearrange("(k p) s -> p k s", p=128)[:, ks])
            nc.scalar.dma_start(
                wv_b[:].rearrange("p (k c) -> p k c", c=CPB),
                w_in(wvT, slice(0, CPB)))
            for q in range(1, 4):
                qs = slice(q * 512, (q + 1) * 512)
                nc.sync.dma_start(
                    xbig[:].rearrange("p (k s) -> p k s", s=S)[:, :, qs],
                    xT[:, qs].rearrange("(k p) s -> p k s", p=128))
            nc.scalar.dma_start(
                wq_m[1][:].rearrange("p (k c) -> p k c", c=128),
                w_in(wqT, slice(128, 256)))
            nc.scalar.dma_start(
                wk_m[1][:].rearrange("p (k c) -> p k c", c=128),
                w_in(wkT, slice(128, 256)))
            nc.scalar.dma_start(
                wp_b[:].rearrange("p (k c) -> p k c", c=C),
                wpT[:, :].rearrange("(k p) c -> p k c", p=128))

            # ---- constants / warmup ----
            junk = small_pool.tile([128, 512], bf16, tag="junk")
            nc.vector.memset(junk[:], 0.5)
            jexp = small_pool.tile([128, 8], bf16, tag="jexp")
            nc.scalar.activation(jexp[:], junk[:, 0:8], EXP, scale=0.25)
            for _ in range(14):
                wps = ps_sm.tile([128, 512], f32, tag="sm", name="warm")
                nc.tensor.matmul(wps[:], junk[:, 0:128], junk[:],
                                 start=True, stop=True)
            ident = small_pool.tile([128, 128], bf16, tag="ident")
            make_identity(nc, ident[:])

            # ---- persistent tiles ----
            qk_tiles = {}
            for m in range(2):
                for nm in ("q", "k"):
                    qk_tiles[(nm, m)] = qk_pool.tile(
                        [128, S], bf16, tag=f"{nm}{m}", name=f"{nm}T{m}")
            va = [va_pool.tile([128, HPC * 65], bf16, tag="va",
                               name=f"va{sc}") for sc in range(SC)]
            for sc in range(SC):
                nc.gpsimd.memset(
                    va[sc][:].rearrange("p (h x) -> p h x", x=65)[:, :, 64:65],
                    1.0)
            on_tiles = [small_pool.tile([128, CPB], bf16, tag="on",
                                        bufs=SC, name=f"on{sc}")
                        for sc in range(SC)]
            ot_tiles = [ot_pool.tile([128, S], bf16, tag=f"ot{m}",
                                     name=f"oT{m}") for m in range(2)]
            exp_tiles = {}
            rp_tiles = {}

            # ---- micro-units ----
            def qk_proj(m, nm, n):
                """q or k projection for head-pair m, token chunk n (512)."""
                wts = wq_m[m] if nm == "q" else wk_m[m]
                dst = qk_tiles[(nm, m)]
                ps = ps_sm.tile([128, 512], f32, tag="sm", name="psqk")
                for kk in range(KC):
                    k = (kk + n * 2) % KC
                    nc.tensor.matmul(
                        ps[:],
                        wts[:, k * 128:(k + 1) * 128],
                        xts(k)[:, n * 512:(n + 1) * 512],
                        start=(kk == 0), stop=(kk == KC - 1))
                nc.vector.tensor_copy(dst[:, n * 512:(n + 1) * 512], ps[:])

            def emit_va(sc):
                """v projection for token chunk sc (128); ones col pre-set."""
                t = va[sc]
                ps = ps_sm.tile([128, CPB], f32, tag="sm", name="psv")
                for k in range(KC):
                    nc.tensor.matmul(
                        ps[:],
                        xts(k)[:, sc * 128:(sc + 1) * 128],
                        wv_b[:, k * CPB:(k + 1) * CPB],
                        start=(k == 0), stop=(k == KC - 1))
                tv = t[:].rearrange("p (h x) -> p h x", x=65)[:, :, 0:64]
                pv_ = ps[:].rearrange("p (h d) -> p h d", d=64)
                nc.vector.tensor_copy(tv, pv_)

            def sblock(m, qv, kv):
                """one score block (4 row-packed matmuls) + its exp."""
                kT_m = qk_tiles[("k", m)]
                qT_m = qk_tiles[("q", m)]
                qs = slice(qv * 256, (qv + 1) * 256)
                pss = ps_big.tile([128, 1024], f32, tag="big", name="pss")
                for j in range(2):
                    kc = 2 * kv + j
                    for h in range(2):
                        nc.tensor.matmul(
                            pss[:, (2 * h + j) * 256:(2 * h + j + 1) * 256],
                            kT_m[64 * h:64 * (h + 1),
                                 kc * 128:(kc + 1) * 128],
                            qT_m[64 * h:64 * (h + 1), qs],
                            start=True, stop=True)
                et = exp_pool.tile([128, 1024], bf16, tag="exp")
                nc.scalar.activation(et[:], pss[:], EXP, scale=float(SCALE))
                exp_tiles[(m, qv, kv)] = et

            def pvg(m, qv, h, qc):
                """one PV accumulation group + normalize."""
                kvs = _allowed(qv)
                hh = 2 * m + h
                g = 2 * h + qc
                if (m, qv) not in rp_tiles:
                    rp_tiles[(m, qv)] = small_pool.tile(
                        [128, 4], f32, tag="rp", bufs=4, name="rp")
                rp = rp_tiles[(m, qv)]
                pg = ps_sm.tile([128, 65], f32, tag="sm", name=f"pg{g}")
                for i, kv in enumerate(kvs):
                    et = exp_tiles[(m, qv, kv)]
                    for j in range(2):
                        kc = 2 * kv + j
                        nc.tensor.matmul(
                            pg[:],
                            et[:, (2 * h + j) * 256 + qc * 128:
                               (2 * h + j) * 256 + qc * 128 + 128],
                            va[kc][:, hh * 65:(hh + 1) * 65],
                            start=(i == 0 and j == 0),
                            stop=(i == len(kvs) - 1 and j == 1))
                sc = qv * 2 + qc
                nc.vector.reciprocal(rp[:, g:g + 1], pg[:, 64:65])
                nc.vector.tensor_scalar_mul(
                    on_tiles[sc][:, hh * 64:(hh + 1) * 64],
                    pg[:, 0:64],
                    rp[:, g:g + 1])

            def transp(half, sc):
                """attention out natural [q, d] -> outT [d, q] chunk."""
                pt = ps_sm.tile([128, 128], bf16, tag="sm", name="pt")
                nc.tensor.transpose(
                    pt[:], on_tiles[sc][:, half * 128:(half + 1) * 128],
                    ident[:])
                nc.vector.tensor_copy(
                    ot_tiles[half][:, sc * 128:(sc + 1) * 128], pt[:])

            def yproj(sc):
                """output projection row chunk sc: y[sc] = outT[:,sc].T @ wpT."""
                ys = ysb_pool.tile([128, C], bf16, tag="ysb")
                for n in range(2):
                    ps = ps_sm.tile([128, 512], f32, tag="sm", name="psy")
                    for k in range(2):
                        nc.tensor.matmul(
                            ps[:],
                            ot_tiles[k][:, sc * 128:(sc + 1) * 128],
                            wp_b[:, k * C + n * 512:k * C + (n + 1) * 512],
                            start=(k == 0), stop=(k == 1))
                    nc.vector.tensor_copy(ys[:, n * 512:(n + 1) * 512],
                                          ps[:])
                nc.sync.dma_start(y[sc * 128:(sc + 1) * 128, :], ys[:])

            # ---- filler FIFO: units with rough PE-time estimates (us) ----
            fifo = deque()

            def q_va(sc):
                fifo.append((0.9, lambda: emit_va(sc)))

            def q_pv(m, qv):
                c = 0.07 * 2 * len(_allowed(qv)) + 0.1
                for h in range(2):
                    for qc in range(2):
                        fifo.append(
                            (c, lambda m=m, qv=qv, h=h, qc=qc:
                             pvg(m, qv, h, qc)))

            def q_transp(half, sc):
                fifo.append((0.4, lambda: transp(half, sc)))

            def q_yproj(sc):
                fifo.append((0.55, lambda: yproj(sc)))

            def pop(min_us):
                t = 0.0
                while fifo and t < min_us:
                    c, fn = fifo.popleft()
                    fn()
                    t += c

            def sb(m, qv):
                """all score blocks of (m, qv), a filler pop between each."""
                for kv in _allowed(qv):
                    sblock(m, qv, kv)
                    pop(0.55)

            # ---- pipelined emission ----
            qk_proj(0, "q", 0); qk_proj(0, "k", 0)
            sb(0, 0); sb(0, 1)
            qk_proj(0, "q", 1); qk_proj(0, "k", 1)
            q_va(2); q_va(3); q_va(0); q_va(1); q_va(4)
            sb(0, 2)
            q_pv(0, 0); q_va(5); q_va(6)
            sb(0, 3)
            q_pv(0, 1); q_transp(0, 0); q_transp(0, 1)
            q_va(7); q_va(8); q_va(9)
            qk_proj(0, "q", 2); qk_proj(0, "k", 2)
            sb(0, 4)
            q_pv(0, 2); q_transp(0, 2); q_transp(0, 3)
            q_va(10); q_va(11); q_va(12)
            qk_proj(0, "q", 3); qk_proj(0, "k", 3)
            sb(0, 5)
            q_pv(0, 3); q_transp(0, 4); q_transp(0, 5)
            q_va(13); q_va(14); q_va(15)
            sb(0, 6)
            q_pv(0, 4); q_transp(0, 6); q_transp(0, 7)
            sb(0, 7)
            q_pv(0, 5); q_transp(0, 8); q_transp(0, 9)
            qk_proj(1, "q", 0); qk_proj(1, "k", 0)
            pop(1.5)
            qk_proj(1, "q", 1); qk_proj(1, "k", 1)
            q_pv(0, 6); q_transp(0, 10); q_transp(0, 11)
            pop(1.5)
            qk_proj(1, "q", 2); qk_proj(1, "k", 2)
            q_pv(0, 7); q_transp(0, 12); q_transp(0, 13)
            pop(1.5)
            qk_proj(1, "q", 3); qk_proj(1, "k", 3)
            q_transp(0, 14); q_transp(0, 15)
            sb(1, 0); sb(1, 1)
            q_pv(1, 0); q_transp(1, 0); q_transp(1, 1)
            sb(1, 2)
            q_pv(1, 1); q_transp(1, 2); q_transp(1, 3)
            q_yproj(0); q_yproj(1)
            sb(1, 3)
            q_pv(1, 2); q_transp(1, 4); q_transp(1, 5)
            q_yproj(2); q_yproj(3)
            sb(1, 4)
            q_pv(1, 3); q_transp(1, 6); q_transp(1, 7)
            q_yproj(4); q_yproj(5)
            sb(1, 5)
            q_pv(1, 4); q_transp(1, 8); q_transp(1, 9)
            q_yproj(6); q_yproj(7)
            sb(1, 6)
            q_pv(1, 5); q_transp(1, 10); q_transp(1, 11)
            q_yproj(8); q_yproj(9)
            sb(1, 7)
            q_pv(1, 6); q_transp(1, 12); q_transp(1, 13)
            q_yproj(10); q_yproj(11)
            q_pv(1, 7); q_yproj(12); q_yproj(13)
            q_transp(1, 14); q_transp(1, 15)
            q_yproj(14); q_yproj(15)
            pop(1e9)

    nc.compile()
    return nc


def _get_compiled():
    if "nc" not in _compiled:
        _compiled["nc"] = build()
    return _compiled["nc"]


def make_in_maps(x, Wq, Wk, Wv, Wp):
    xf = np.asarray(x, np.float32).reshape(B, S, C)
    in_maps = []
    for c in range(N_CORES):
        b, g = divmod(c, HPC)
        hs = slice(g * CPB, (g + 1) * CPB)
        bf = ml_dtypes.bfloat16
        in_maps.append({
            "xT": np.ascontiguousarray(xf[b].T).astype(bf),
            "wqT": np.ascontiguousarray(np.asarray(Wq, np.float32)[hs].T).astype(bf),
            "wkT": np.ascontiguousarray(np.asarray(Wk, np.float32)[hs].T).astype(bf),
            "wvT": np.ascontiguousarray(np.asarray(Wv, np.float32)[hs].T).astype(bf),
            "wpT": np.ascontiguousarray(np.asarray(Wp, np.float32)[:, hs].T).astype(bf),
        })
    return in_maps


def kernel(x, Wq, Wk, Wv, Wp, bp, _trace=False, _tmpdir=None):
    global LAST_RESULTS
    from concourse import bass_utils

    nc = _get_compiled()
    in_maps = make_in_maps(x, Wq, Wk, Wv, Wp)
    kwargs = {}
    if _trace:
        kwargs = {"trace": True, "tmpdir": _tmpdir}
    res = bass_utils.run_bass_kernel_spmd(
        nc, in_maps, core_ids=list(range(N_CORES)), **kwargs)
    LAST_RESULTS = res
    yout = np.zeros((B, S, C), np.float32)
    for c in range(N_CORES):
        yout[c // HPC] += res.results[c]["y"].astype(np.float32)
    yout += np.asarray(bp, np.float32).reshape(1, 1, C)
    return yout.reshape(B, V, L, C)


# revision 16
# speedup vs baseline: 1.0244x; 1.0244x over previous
"""Block-sparse (view-causal) multi-head attention on 8 TRN2 NeuronCores.

Full inputs in, full output out. Sharding: data-parallel over batch (B=2),
tensor-parallel over heads (16 heads -> 4 per core). Each core computes its
4 heads' attention + its slice of the output projection; the host sums the
4 head-group partial projections per batch (the tensor-parallel reduce).

v5: micro-unit software pipelining. The emission stream alternates single
score blocks (4 matmuls + exp) with ~0.5us filler units (v-projection
chunks, the second head-pair's q/k projection, PV groups, transposes,
output-projection chunks) pulled from a FIFO, so the PE never stalls on
the 2-deep score-PSUM ring waiting for the ACT exp drain, and the ACT exp
stream starts as soon as the first score block exists (~15us). Input DMA
is batched into a few large strided transfers, first token-quarter first.

Device-side layout: activations kept transposed (qT/kT [dh, tokens]) so the
score matmuls need no transposes and the two K=64 head matmuls row-pack in
the PE array; V is augmented with a ones column so each PV matmul
accumulates both sum(exp*v) and the softmax denominator in PSUM. All
matmul operands are bf16 (PSUM accumulation in fp32).
"""

import sys

if "/opt/trn_rl_repo" not in sys.path:
    sys.path.insert(0, "/opt/trn_rl_repo")

import numpy as np
import ml_dtypes
from collections import deque

B, V, L, C, H = 2, 8, 256, 1024, 16
S = V * L                # 2048 tokens
DH = C // H              # 64
HPC = 4                  # heads per core
CPB = HPC * DH           # 256 channel block per core
N_CORES = 8
SCALE = DH ** -0.5       # 1/8, folded into the exp activation

_compiled = {}
LAST_RESULTS = None
PACK_QK = True           # kept for test.py compat
SAFE_RECIP = False
SPLIT_ACT = False

KC = C // 128            # 8 contraction chunks for the projections
SC = S // 128            # 16 sequence chunks


def _allowed(qv):
    """View-level mask row: views 0/1 cross-attend only; views >=2 block-causal."""
    if qv == 0:
        return [1]
    if qv == 1:
        return [0]
    return list(range(qv + 1))


def build():
    import concourse.tile as tile
    from concourse import bacc, mybir
    from concourse.masks import make_identity

    f32 = mybir.dt.float32
    bf16 = mybir.dt.bfloat16
    EXP = mybir.ActivationFunctionType.Exp

    nc = bacc.Bacc("TRN2", target_bir_lowering=False, debug=False,
                   num_devices=N_CORES)
    xT = nc.dram_tensor("xT", [C, S], bf16, kind="ExternalInput").ap()
    wqT = nc.dram_tensor("wqT", [C, CPB], bf16, kind="ExternalInput").ap()
    wkT = nc.dram_tensor("wkT", [C, CPB], bf16, kind="ExternalInput").ap()
    wvT = nc.dram_tensor("wvT", [C, CPB], bf16, kind="ExternalInput").ap()
    wpT = nc.dram_tensor("wpT", [CPB, C], bf16, kind="ExternalInput").ap()
    y = nc.dram_tensor("y", [S, C], bf16, kind="ExternalOutput").ap()

    with tile.TileContext(nc) as tc:
        with (
            tc.tile_pool(name="xt", bufs=1) as xt_pool,
            tc.tile_pool(name="wts", bufs=1) as w_pool,
            tc.tile_pool(name="qk", bufs=1) as qk_pool,
            tc.tile_pool(name="va", bufs=SC) as va_pool,
            tc.tile_pool(name="ot", bufs=1) as ot_pool,
            tc.tile_pool(name="exp", bufs=22) as exp_pool,
            tc.tile_pool(name="small", bufs=1) as small_pool,
            tc.tile_pool(name="ysb", bufs=4) as ysb_pool,
            tc.tile_pool(name="psb", bufs=2, space="PSUM") as ps_big,
            tc.tile_pool(name="pss", bufs=4, space="PSUM") as ps_sm,
        ):
            # ---- input DMAs: few large strided transfers ----
            wq_m, wk_m = [], []
            for m in range(2):
                tq = w_pool.tile([128, KC * 128], bf16, tag=f"wq{m}",
                                 name=f"wq{m}")
                tk = w_pool.tile([128, KC * 128], bf16, tag=f"wk{m}",
                                 name=f"wk{m}")
                wq_m.append(tq)
                wk_m.append(tk)
            wv_b = w_pool.tile([128, KC * CPB], bf16, tag="wv", name="wv")
            wp_b = w_pool.tile([128, 2 * C], bf16, tag="wp", name="wp")

            def w_in(dram_ap, cols):
                return dram_ap[:, cols].rearrange("(k p) c -> p k c", p=128)

            xbig = xt_pool.tile([128, KC * S], bf16, tag="xt", name="xbig")

            def xts(k):
                return xbig[:, k * S:(k + 1) * S]

            # weights on the scalar ring (m0 q/k halves first); x on the sync
            # ring with the first token-quarter (in two chunk-halves) first.
            for m in (0, 1):
                ms = slice(m * 128, (m + 1) * 128)
                nc.scalar.dma_start(
                    wq_m[m][:].rearrange("p (k c) -> p k c", c=128),
                    w_in(wqT, ms))
                nc.scalar.dma_start(
                    wk_m[m][:].rearrange("p (k c) -> p k c", c=128),
                    w_in(wkT, ms))
                if m == 0:
                    nc.scalar.dma_start(
                        wv_b[:].rearrange("p (k c) -> p k c", c=CPB),
                        w_in(wvT, slice(0, CPB)))
            nc.scalar.dma_start(
                wp_b[:].rearrange("p (k c) -> p k c", c=C),
                wpT[:, :].rearrange("(k p) c -> p k c", p=128))
            for ks in (slice(0, 4), slice(4, 8)):
                nc.sync.dma_start(
                    xbig[:].rearrange("p (k s) -> p k s", s=S)[:, ks, 0:512],
                    xT[:, 0:512].rearrange("(k p) s -> p k s", p=128)[:, ks])
            for q in range(1, 4):
                qs = slice(q * 512, (q + 1) * 512)
                nc.sync.dma_start(
                    xbig[:].rearrange("p (k s) -> p k s", s=S)[:, :, qs],
                    xT[:, qs].rearrange("(k p) s -> p k s", p=128))

            # ---- constants / warmup ----
            junk = small_pool.tile([128, 512], bf16, tag="junk")
            nc.vector.memset(junk[:], 0.5)
            jexp = small_pool.tile([128, 8], bf16, tag="jexp")
            nc.scalar.activation(jexp[:], junk[:, 0:8], EXP, scale=0.25)
            for _ in range(17):
                wps = ps_sm.tile([128, 512], f32, tag="sm", name="warm")
                nc.tensor.matmul(wps[:], junk[:, 0:128], junk[:],
                                 start=True, stop=True)
            ident = small_pool.tile([128, 128], bf16, tag="ident")
            make_identity(nc, ident[:])

            # ---- persistent tiles ----
            qk_tiles = {}
            for m in range(2):
                for nm in ("q", "k"):
                    qk_tiles[(nm, m)] = qk_pool.tile(
                        [128, S], bf16, tag=f"{nm}{m}", name=f"{nm}T{m}")
            va = [va_pool.tile([128, HPC * 65], bf16, tag="va",
                               name=f"va{sc}") for sc in range(SC)]
            for sc in range(SC):
                nc.gpsimd.memset(
                    va[sc][:].rearrange("p (h x) -> p h x", x=65)[:, :, 64:65],
                    1.0)
            on_tiles = [small_pool.tile([128, CPB], bf16, tag="on",
                                        bufs=SC, name=f"on{sc}")
                        for sc in range(SC)]
            ot_tiles = [ot_pool.tile([128, S], bf16, tag=f"ot{m}",
                                     name=f"oT{m}") for m in range(2)]
            exp_tiles = {}
            rp_tiles = {}

            # ---- micro-units ----
            def qk_proj(m, nm, n):
                """q or k projection for head-pair m, token chunk n (512)."""
                wts = wq_m[m] if nm == "q" else wk_m[m]
                dst = qk_tiles[(nm, m)]
                ps = ps_sm.tile([128, 512], f32, tag="sm", name="psqk")
                for kk in range(KC):
                    k = (kk + n * 2) % KC
                    nc.tensor.matmul(
                        ps[:],
                        wts[:, k * 128:(k + 1) * 128],
                        xts(k)[:, n * 512:(n + 1) * 512],
                        start=(kk == 0), stop=(kk == KC - 1))
                nc.vector.tensor_copy(dst[:, n * 512:(n + 1) * 512], ps[:])

            def emit_va(sc):
                """v projection for token chunk sc (128); ones col pre-set."""
                t = va[sc]
                ps = ps_sm.tile([128, CPB], f32, tag="sm", name="psv")
                for k in range(KC):
                    nc.tensor.matmul(
                        ps[:],
                        xts(k)[:, sc * 128:(sc + 1) * 128],
                        wv_b[:, k * CPB:(k + 1) * CPB],
                        start=(k == 0), stop=(k == KC - 1))
                tv = t[:].rearrange("p (h x) -> p h x", x=65)[:, :, 0:64]
                pv_ = ps[:].rearrange("p (h d) -> p h d", d=64)
                nc.vector.tensor_copy(tv, pv_)

            def sblock(m, qv, kv):
                """one score block (4 row-packed matmuls) + its exp."""
                kT_m = qk_tiles[("k", m)]
                qT_m = qk_tiles[("q", m)]
                qs = slice(qv * 256, (qv + 1) * 256)
                pss = ps_big.tile([128, 1024], f32, tag="big", name="pss")
                for j in range(2):
                    kc = 2 * kv + j
                    for h in range(2):
                        nc.tensor.matmul(
                            pss[:, (2 * h + j) * 256:(2 * h + j + 1) * 256],
                            kT_m[64 * h:64 * (h + 1),
                                 kc * 128:(kc + 1) * 128],
                            qT_m[64 * h:64 * (h + 1), qs],
                            start=True, stop=True)
                et = exp_pool.tile([128, 1024], bf16, tag="exp")
                nc.scalar.activation(et[:], pss[:], EXP, scale=float(SCALE))
                exp_tiles[(m, qv, kv)] = et

            def pvg(m, qv, h, qc):
                """one PV accumulation group + normalize."""
                kvs = _allowed(qv)
                hh = 2 * m + h
                g = 2 * h + qc
                if (m, qv) not in rp_tiles:
                    rp_tiles[(m, qv)] = small_pool.tile(
                        [128, 4], f32, tag="rp", bufs=4, name="rp")
                rp = rp_tiles[(m, qv)]
                pg = ps_sm.tile([128, 65], f32, tag="sm", name=f"pg{g}")
                for i, kv in enumerate(kvs):
                    et = exp_tiles[(m, qv, kv)]
                    for j in range(2):
                        kc = 2 * kv + j
                        nc.tensor.matmul(
                            pg[:],
                            et[:, (2 * h + j) * 256 + qc * 128:
                               (2 * h + j) * 256 + qc * 128 + 128],
                            va[kc][:, hh * 65:(hh + 1) * 65],
                            start=(i == 0 and j == 0),
                            stop=(i == len(kvs) - 1 and j == 1))
                sc = qv * 2 + qc
                nc.vector.reciprocal(rp[:, g:g + 1], pg[:, 64:65])
                nc.vector.tensor_scalar_mul(
                    on_tiles[sc][:, hh * 64:(hh + 1) * 64],
                    pg[:, 0:64],
                    rp[:, g:g + 1])

            def transp(half, sc):
                """attention out natural [q, d] -> outT [d, q] chunk."""
                pt = ps_sm.tile([128, 128], bf16, tag="sm", name="pt")
                nc.tensor.transpose(
                    pt[:], on_tiles[sc][:, half * 128:(half + 1) * 128],
                    ident[:])
                nc.vector.tensor_copy(
                    ot_tiles[half][:, sc * 128:(sc + 1) * 128], pt[:])

            def yproj(sc):
                """output projection row chunk sc: y[sc] = outT[:,sc].T @ wpT."""
                ys = ysb_pool.tile([128, C], bf16, tag="ysb")
                for n in range(2):
                    ps = ps_sm.tile([128, 512], f32, tag="sm", name="psy")
                    for k in range(2):
                        nc.tensor.matmul(
                            ps[:],
                            ot_tiles[k][:, sc * 128:(sc + 1) * 128],
                            wp_b[:, k * C + n * 512:k * C + (n + 1) * 512],
                            start=(k == 0), stop=(k == 1))
                    nc.vector.tensor_copy(ys[:, n * 512:(n + 1) * 512],
                                          ps[:])
                nc.sync.dma_start(y[sc * 128:(sc + 1) * 128, :], ys[:])

            # ---- filler FIFO: units with rough PE-time estimates (us) ----
            fifo = deque()

            def q_va(sc):
                fifo.append((0.9, lambda: emit_va(sc)))

            def q_pv(m, qv):
                c = 0.07 * 2 * len(_allowed(qv)) + 0.1
                for h in range(2):
                    for qc in range(2):
                        fifo.append(
                            (c, lambda m=m, qv=qv, h=h, qc=qc:
                             pvg(m, qv, h, qc)))

            def q_transp(half, sc):
                fifo.append((0.4, lambda: transp(half, sc)))

            def q_yproj(sc):
                fifo.append((0.55, lambda: yproj(sc)))

            def pop(min_us):
                t = 0.0
                while fifo and t < min_us:
                    c, fn = fifo.popleft()
                    fn()
                    t += c

            def sb(m, qv):
                """all score blocks of (m, qv), a filler pop between each."""
                for kv in _allowed(qv):
                    sblock(m, qv, kv)
                    pop(0.55)

            # ---- pipelined emission ----
            qk_proj(0, "q", 0); qk_proj(0, "k", 0)
            sb(0, 0); sb(0, 1)
            qk_proj(0, "q", 1); qk_proj(0, "k", 1)
            q_va(2); q_va(3); q_va(0); q_va(1); q_va(4)
            sb(0, 2)
            q_pv(0, 0); q_va(5); q_va(6)
            sb(0, 3)
            q_pv(0, 1); q_transp(0, 0); q_transp(0, 1)
            q_va(7); q_va(8); q_va(9)
            qk_proj(0, "q", 2); qk_proj(0, "k", 2)
            sb(0, 4)
            q_pv(0, 2); q_transp(0, 2); q_transp(0, 3)
            q_va(10); q_va(11); q_va(12)
            qk_proj(0, "q", 3); qk_proj(0, "k", 3)
            sb(0, 5)
            q_pv(0, 3); q_transp(0, 4); q_transp(0, 5)
            q_va(13); q_va(14); q_va(15)
            sb(0, 6)
            q_pv(0, 4); q_transp(0, 6); q_transp(0, 7)
            sb(0, 7)
            q_pv(0, 5); q_transp(0, 8); q_transp(0, 9)
            qk_proj(1, "q", 0); qk_proj(1, "k", 0)
            pop(1.5)
            qk_proj(1, "q", 1); qk_proj(1, "k", 1)
            q_pv(0, 6); q_transp(0, 10); q_transp(0, 11)
            pop(1.5)
            qk_proj(1, "q", 2); qk_proj(1, "k", 2)
            q_pv(0, 7); q_transp(0, 12); q_transp(0, 13)
            pop(1.5)
            qk_proj(1, "q", 3); qk_proj(1, "k", 3)
            q_transp(0, 14); q_transp(0, 15)
            sb(1, 0); sb(1, 1)
            q_pv(1, 0); q_transp(1, 0); q_transp(1, 1)
            sb(1, 2)
            q_pv(1, 1); q_transp(1, 2); q_transp(1, 3)
            q_yproj(0); q_yproj(1)
            sb(1, 3)
            q_pv(1, 2); q_transp(1, 4); q_transp(1, 5)
            q_yproj(2); q_yproj(3)
            sb(1, 4)
            q_pv(1, 3); q_transp(1, 6); q_transp(1, 7)
            q_yproj(4); q_yproj(5)
            sb(1, 5)
            q_pv(1, 4); q_transp(1, 8); q_transp(1, 9)
            q_yproj(6); q_yproj(7)
            sb(1, 6)
            q_pv(1, 5); q_transp(1, 10); q_transp(1, 11)
            q_yproj(8); q_yproj(9)
            sb(1, 7)
            q_pv(1, 6); q_transp(1, 12); q_transp(1, 13)
            q_yproj(10); q_yproj(11)
            q_pv(1, 7); q_yproj(12); q_yproj(13)
            q_transp(1, 14); q_transp(1, 15)
            q_yproj(14); q_yproj(15)
            pop(1e9)

    nc.compile()
    return nc


def _get_compiled():
    if "nc" not in _compiled:
        _compiled["nc"] = build()
    return _compiled["nc"]


def make_in_maps(x, Wq, Wk, Wv, Wp):
    xf = np.asarray(x, np.float32).reshape(B, S, C)
    in_maps = []
    for c in range(N_CORES):
        b, g = divmod(c, HPC)
        hs = slice(g * CPB, (g + 1) * CPB)
        bf = ml_dtypes.bfloat16
        in_maps.append({
            "xT": np.ascontiguousarray(xf[b].T).astype(bf),
            "wqT": np.ascontiguousarray(np.asarray(Wq, np.float32)[hs].T).astype(bf),
            "wkT": np.ascontiguousarray(np.asarray(Wk, np.float32)[hs].T).astype(bf),
            "wvT": np.ascontiguousarray(np.asarray(Wv, np.float32)[hs].T).astype(bf),
            "wpT": np.ascontiguousarray(np.asarray(Wp, np.float32)[:, hs].T).astype(bf),
        })
    return in_maps


def kernel(x, Wq, Wk, Wv, Wp, bp, _trace=False, _tmpdir=None):
    global LAST_RESULTS
    from concourse import bass_utils

    nc = _get_compiled()
    in_maps = make_in_maps(x, Wq, Wk, Wv, Wp)
    kwargs = {}
    if _trace:
        kwargs = {"trace": True, "tmpdir": _tmpdir}
    res = bass_utils.run_bass_kernel_spmd(
        nc, in_maps, core_ids=list(range(N_CORES)), **kwargs)
    LAST_RESULTS = res
    yout = np.zeros((B, S, C), np.float32)
    for c in range(N_CORES):
        yout[c // HPC] += res.results[c]["y"].astype(np.float32)
    yout += np.asarray(bp, np.float32).reshape(1, 1, C)
    return yout.reshape(B, V, L, C)


# revision 21
# speedup vs baseline: 1.0361x; 1.0115x over previous
"""Block-sparse (view-causal) multi-head attention on 8 TRN2 NeuronCores.

Full inputs in, full output out. Sharding: data-parallel over batch (B=2),
tensor-parallel over heads (16 heads -> 4 per core). Each core computes its
4 heads' attention + its slice of the output projection; the host sums the
4 head-group partial projections per batch (the tensor-parallel reduce).

v5: micro-unit software pipelining. The emission stream alternates single
score blocks (4 matmuls + exp) with ~0.5us filler units (v-projection
chunks, the second head-pair's q/k projection, PV groups, transposes,
output-projection chunks) pulled from a FIFO, so the PE never stalls on
the 2-deep score-PSUM ring waiting for the ACT exp drain, and the ACT exp
stream starts as soon as the first score block exists (~15us). Input DMA
is batched into a few large strided transfers, first token-quarter first.

Device-side layout: activations kept transposed (qT/kT [dh, tokens]) so the
score matmuls need no transposes and the two K=64 head matmuls row-pack in
the PE array; V is augmented with a ones column so each PV matmul
accumulates both sum(exp*v) and the softmax denominator in PSUM. All
matmul operands are bf16 (PSUM accumulation in fp32).
"""

import sys

if "/opt/trn_rl_repo" not in sys.path:
    sys.path.insert(0, "/opt/trn_rl_repo")

import numpy as np
import ml_dtypes
from collections import deque

B, V, L, C, H = 2, 8, 256, 1024, 16
S = V * L                # 2048 tokens
DH = C // H              # 64
HPC = 4                  # heads per core
CPB = HPC * DH           # 256 channel block per core
N_CORES = 8
SCALE = DH ** -0.5       # 1/8, folded into the exp activation

_compiled = {}
LAST_RESULTS = None
PACK_QK = True           # kept for test.py compat
SAFE_RECIP = False
SPLIT_ACT = False

KC = C // 128            # 8 contraction chunks for the projections
SC = S // 128            # 16 sequence chunks


def _allowed(qv):
    """View-level mask row: views 0/1 cross-attend only; views >=2 block-causal."""
    if qv == 0:
        return [1]
    if qv == 1:
        return [0]
    return list(range(qv + 1))


def build():
    import concourse.tile as tile
    from concourse import bacc, mybir
    from concourse.masks import make_identity

    f32 = mybir.dt.float32
    bf16 = mybir.dt.bfloat16
    EXP = mybir.ActivationFunctionType.Exp

    nc = bacc.Bacc("TRN2", target_bir_lowering=False, debug=False,
                   num_devices=N_CORES)
    xT = nc.dram_tensor("xT", [C, S], bf16, kind="ExternalInput").ap()
    wqT = nc.dram_tensor("wqT", [C, CPB], bf16, kind="ExternalInput").ap()
    wkT = nc.dram_tensor("wkT", [C, CPB], bf16, kind="ExternalInput").ap()
    wvT = nc.dram_tensor("wvT", [C, CPB], bf16, kind="ExternalInput").ap()
    wpT = nc.dram_tensor("wpT", [CPB, C], bf16, kind="ExternalInput").ap()
    y = nc.dram_tensor("y", [S, C], bf16, kind="ExternalOutput").ap()

    with tile.TileContext(nc) as tc:
        with (
            tc.tile_pool(name="xt", bufs=1) as xt_pool,
            tc.tile_pool(name="wts", bufs=1) as w_pool,
            tc.tile_pool(name="qk", bufs=1) as qk_pool,
            tc.tile_pool(name="va", bufs=SC) as va_pool,
            tc.tile_pool(name="ot", bufs=1) as ot_pool,
            tc.tile_pool(name="exp", bufs=22) as exp_pool,
            tc.tile_pool(name="small", bufs=1) as small_pool,
            tc.tile_pool(name="ysb", bufs=4) as ysb_pool,
            tc.tile_pool(name="psb", bufs=2, space="PSUM") as ps_big,
            tc.tile_pool(name="pss", bufs=4, space="PSUM") as ps_sm,
        ):
            # ---- input DMAs: few large strided transfers ----
            wq_m, wk_m = [], []
            for m in range(2):
                tq = w_pool.tile([128, KC * 128], bf16, tag=f"wq{m}",
                                 name=f"wq{m}")
                tk = w_pool.tile([128, KC * 128], bf16, tag=f"wk{m}",
                                 name=f"wk{m}")
                wq_m.append(tq)
                wk_m.append(tk)
            wv_b = w_pool.tile([128, KC * CPB], bf16, tag="wv", name="wv")
            wp_b = w_pool.tile([128, 2 * C], bf16, tag="wp", name="wp")

            def w_in(dram_ap, cols):
                return dram_ap[:, cols].rearrange("(k p) c -> p k c", p=128)

            xbig = xt_pool.tile([128, KC * S], bf16, tag="xt", name="xbig")

            def xts(k):
                return xbig[:, k * S:(k + 1) * S]

            # weights on the scalar ring (m0 q/k halves first); x on the sync
            # ring with the first token-quarter (in two chunk-halves) first.
            for m in (0, 1):
                ms = slice(m * 128, (m + 1) * 128)
                nc.scalar.dma_start(
                    wq_m[m][:].rearrange("p (k c) -> p k c", c=128),
                    w_in(wqT, ms))
                nc.scalar.dma_start(
                    wk_m[m][:].rearrange("p (k c) -> p k c", c=128),
                    w_in(wkT, ms))
                if m == 0:
                    nc.scalar.dma_start(
                        wv_b[:].rearrange("p (k c) -> p k c", c=CPB),
                        w_in(wvT, slice(0, CPB)))
            nc.scalar.dma_start(
                wp_b[:].rearrange("p (k c) -> p k c", c=C),
                wpT[:, :].rearrange("(k p) c -> p k c", p=128))
            for ks in (slice(0, 4), slice(4, 8)):
                nc.sync.dma_start(
                    xbig[:].rearrange("p (k s) -> p k s", s=S)[:, ks, 0:512],
                    xT[:, 0:512].rearrange("(k p) s -> p k s", p=128)[:, ks])
            for q in range(1, 4):
                qs = slice(q * 512, (q + 1) * 512)
                nc.sync.dma_start(
                    xbig[:].rearrange("p (k s) -> p k s", s=S)[:, :, qs],
                    xT[:, qs].rearrange("(k p) s -> p k s", p=128))

            # ---- constants / warmup ----
            junk = small_pool.tile([128, 512], bf16, tag="junk")
            nc.vector.memset(junk[:], 0.5)
            jexp = small_pool.tile([128, 8], bf16, tag="jexp")
            nc.scalar.activation(jexp[:], junk[:, 0:8], EXP, scale=0.25)
            for _ in range(17):
                wps = ps_sm.tile([128, 512], f32, tag="sm", name="warm")
                nc.tensor.matmul(wps[:], junk[:, 0:128], junk[:],
                                 start=True, stop=True)
            ident = small_pool.tile([128, 128], bf16, tag="ident")
            make_identity(nc, ident[:])

            # ---- persistent tiles ----
            qk_tiles = {}
            for m in range(2):
                for nm in ("q", "k"):
                    qk_tiles[(nm, m)] = qk_pool.tile(
                        [128, S], bf16, tag=f"{nm}{m}", name=f"{nm}T{m}")
            va = [va_pool.tile([128, HPC * 65], bf16, tag="va",
                               name=f"va{sc}") for sc in range(SC)]
            for sc in range(SC):
                nc.gpsimd.memset(
                    va[sc][:].rearrange("p (h x) -> p h x", x=65)[:, :, 64:65],
                    1.0)
            on_tiles = [small_pool.tile([128, CPB], bf16, tag="on",
                                        bufs=SC, name=f"on{sc}")
                        for sc in range(SC)]
            ot_tiles = [ot_pool.tile([128, S], bf16, tag=f"ot{m}",
                                     name=f"oT{m}") for m in range(2)]
            exp_tiles = {}
            rp_tiles = {}

            # ---- micro-units ----
            def qk_proj(m, nm, n):
                """q or k projection for head-pair m, token chunk n (512)."""
                wts = wq_m[m] if nm == "q" else wk_m[m]
                dst = qk_tiles[(nm, m)]
                ps = ps_sm.tile([128, 512], f32, tag="sm", name="psqk")
                for kk in range(KC):
                    k = (kk + n * 2) % KC
                    nc.tensor.matmul(
                        ps[:],
                        wts[:, k * 128:(k + 1) * 128],
                        xts(k)[:, n * 512:(n + 1) * 512],
                        start=(kk == 0), stop=(kk == KC - 1))
                nc.vector.tensor_copy(dst[:, n * 512:(n + 1) * 512], ps[:])

            def emit_va(sc):
                """v projection for token chunk sc (128); ones col pre-set."""
                t = va[sc]
                ps = ps_sm.tile([128, CPB], f32, tag="sm", name="psv")
                for k in range(KC):
                    nc.tensor.matmul(
                        ps[:],
                        xts(k)[:, sc * 128:(sc + 1) * 128],
                        wv_b[:, k * CPB:(k + 1) * CPB],
                        start=(k == 0), stop=(k == KC - 1))
                tv = t[:].rearrange("p (h x) -> p h x", x=65)[:, :, 0:64]
                pv_ = ps[:].rearrange("p (h d) -> p h d", d=64)
                nc.vector.tensor_copy(tv, pv_)

            def sblock(m, qv, kv):
                """one score block (4 row-packed matmuls) + its exp."""
                kT_m = qk_tiles[("k", m)]
                qT_m = qk_tiles[("q", m)]
                qs = slice(qv * 256, (qv + 1) * 256)
                pss = ps_big.tile([128, 1024], f32, tag="big", name="pss")
                for j in range(2):
                    kc = 2 * kv + j
                    for h in range(2):
                        nc.tensor.matmul(
                            pss[:, (2 * h + j) * 256:(2 * h + j + 1) * 256],
                            kT_m[64 * h:64 * (h + 1),
                                 kc * 128:(kc + 1) * 128],
                            qT_m[64 * h:64 * (h + 1), qs],
                            start=True, stop=True)
                et = exp_pool.tile([128, 1024], bf16, tag="exp")
                nc.scalar.activation(et[:], pss[:], EXP, scale=float(SCALE))
                exp_tiles[(m, qv, kv)] = et

            def pvg(m, qv, h, qc, tail=False):
                """one PV accumulation group + normalize."""
                kvs = _allowed(qv)
                hh = 2 * m + h
                g = 2 * h + qc
                if (m, qv) not in rp_tiles:
                    rp_tiles[(m, qv)] = small_pool.tile(
                        [128, 4], f32, tag="rp", bufs=4, name="rp")
                rp = rp_tiles[(m, qv)]
                pg = ps_sm.tile([128, 65], f32, tag="sm", name=f"pg{g}")
                for i, kv in enumerate(kvs):
                    et = exp_tiles[(m, qv, kv)]
                    for j in range(2):
                        kc = 2 * kv + j
                        nc.tensor.matmul(
                            pg[:],
                            et[:, (2 * h + j) * 256 + qc * 128:
                               (2 * h + j) * 256 + qc * 128 + 128],
                            va[kc][:, hh * 65:(hh + 1) * 65],
                            start=(i == 0 and j == 0),
                            stop=(i == len(kvs) - 1 and j == 1))
                sc = qv * 2 + qc
                nc.vector.reciprocal(rp[:, g:g + 1], pg[:, 64:65])
                if tail:
                    # ACT is idle after the last exp; keep DVE off the
                    # critical tail
                    nc.scalar.mul(on_tiles[sc][:, hh * 64:(hh + 1) * 64],
                                  pg[:, 0:64], rp[:, g:g + 1])
                else:
                    nc.vector.tensor_scalar_mul(
                        on_tiles[sc][:, hh * 64:(hh + 1) * 64],
                        pg[:, 0:64],
                        rp[:, g:g + 1])

            def transp(half, sc, tail=False):
                """attention out natural [q, d] -> outT [d, q] chunk."""
                pt = ps_sm.tile([128, 128], bf16, tag="sm", name="pt")
                nc.tensor.transpose(
                    pt[:], on_tiles[sc][:, half * 128:(half + 1) * 128],
                    ident[:])
                dst = ot_tiles[half][:, sc * 128:(sc + 1) * 128]
                if tail:
                    nc.scalar.copy(dst, pt[:])
                else:
                    nc.vector.tensor_copy(dst, pt[:])

            def yproj(sc, tail=False):
                """output projection row chunk sc: y[sc] = outT[:,sc].T @ wpT."""
                ys = ysb_pool.tile([128, C], bf16, tag="ysb")
                for n in range(2):
                    ps = ps_sm.tile([128, 512], f32, tag="sm", name="psy")
                    for k in range(2):
                        nc.tensor.matmul(
                            ps[:],
                            ot_tiles[k][:, sc * 128:(sc + 1) * 128],
                            wp_b[:, k * C + n * 512:k * C + (n + 1) * 512],
                            start=(k == 0), stop=(k == 1))
                    dst = ys[:, n * 512:(n + 1) * 512]
                    if tail and n == 0:
                        nc.scalar.copy(dst, ps[:])
                    else:
                        nc.vector.tensor_copy(dst, ps[:])
                    nc.sync.dma_start(
                        y[sc * 128:(sc + 1) * 128,
                          n * 512:(n + 1) * 512], dst)

            # ---- filler FIFO: units with rough PE-time estimates (us) ----
            fifo = deque()

            def q_va(sc):
                fifo.append((0.9, lambda: emit_va(sc)))

            def q_pv(m, qv, tail=False):
                c = 0.07 * 2 * len(_allowed(qv)) + 0.1
                for h in range(2):
                    for qc in range(2):
                        fifo.append(
                            (c, lambda m=m, qv=qv, h=h, qc=qc:
                             pvg(m, qv, h, qc, tail)))

            def q_transp(half, sc, tail=False):
                fifo.append((0.4, lambda: transp(half, sc, tail)))

            def q_yproj(sc, tail=False):
                fifo.append((0.55, lambda: yproj(sc, tail)))

            def pop(min_us):
                t = 0.0
                while fifo and t < min_us:
                    c, fn = fifo.popleft()
                    fn()
                    t += c

            def sb(m, qv):
                """all score blocks of (m, qv), a filler pop between each."""
                for kv in _allowed(qv):
                    sblock(m, qv, kv)
                    pop(0.55)

            # ---- pipelined emission ----
            qk_proj(0, "q", 0); qk_proj(0, "k", 0)
            sb(0, 0); sb(0, 1)
            qk_proj(0, "q", 1); qk_proj(0, "k", 1)
            q_va(2); q_va(3); q_va(0); q_va(1); q_va(4)
            sb(0, 2)
            q_pv(0, 0); q_va(5); q_va(6)
            sb(0, 3)
            q_pv(0, 1); q_transp(0, 0); q_transp(0, 1)
            q_va(7); q_va(8); q_va(9)
            qk_proj(0, "q", 2); qk_proj(0, "k", 2)
            sb(0, 4)
            q_pv(0, 2); q_transp(0, 2); q_transp(0, 3)
            q_va(10); q_va(11); q_va(12)
            qk_proj(0, "q", 3); qk_proj(0, "k", 3)
            sb(0, 5)
            q_pv(0, 3); q_transp(0, 4); q_transp(0, 5)
            q_va(13); q_va(14); q_va(15)
            sb(0, 6)
            q_pv(0, 4); q_transp(0, 6); q_transp(0, 7)
            sb(0, 7)
            q_pv(0, 5); q_transp(0, 8); q_transp(0, 9)
            qk_proj(1, "q", 0); qk_proj(1, "k", 0)
            pop(1.5)
            qk_proj(1, "q", 1); qk_proj(1, "k", 1)
            q_pv(0, 6); q_transp(0, 10); q_transp(0, 11)
            pop(1.5)
            qk_proj(1, "q", 2); qk_proj(1, "k", 2)
            q_pv(0, 7); q_transp(0, 12); q_transp(0, 13)
            pop(1.5)
            qk_proj(1, "q", 3); qk_proj(1, "k", 3)
            q_transp(0, 14); q_transp(0, 15)
            sb(1, 0); sb(1, 1)
            q_pv(1, 0); q_transp(1, 0); q_transp(1, 1)
            sb(1, 2)
            q_pv(1, 1); q_transp(1, 2); q_transp(1, 3)
            q_yproj(0); q_yproj(1)
            sb(1, 3)
            q_pv(1, 2); q_transp(1, 4); q_transp(1, 5)
            q_yproj(2); q_yproj(3)
            sb(1, 4)
            q_pv(1, 3); q_transp(1, 6); q_transp(1, 7)
            q_yproj(4); q_yproj(5)
            sb(1, 5)
            q_pv(1, 4); q_transp(1, 8); q_transp(1, 9)
            q_yproj(6); q_yproj(7)
            sb(1, 6)
            q_pv(1, 5); q_transp(1, 10); q_transp(1, 11)
            q_yproj(8); q_yproj(9)
            sb(1, 7)
            q_pv(1, 6); q_transp(1, 12); q_transp(1, 13)
            q_yproj(10); q_yproj(11)
            q_pv(1, 7, tail=True); q_yproj(12, tail=True)
            q_yproj(13, tail=True)
            q_transp(1, 14, tail=True); q_transp(1, 15, tail=True)
            q_yproj(14, tail=True); q_yproj(15, tail=True)
            pop(1e9)

    nc.compile()
    return nc


def _get_compiled():
    if "nc" not in _compiled:
        _compiled["nc"] = build()
    return _compiled["nc"]


def make_in_maps(x, Wq, Wk, Wv, Wp):
    xf = np.asarray(x, np.float32).reshape(B, S, C)
    in_maps = []
    for c in range(N_CORES):
        b, g = divmod(c, HPC)
        hs = slice(g * CPB, (g + 1) * CPB)
        bf = ml_dtypes.bfloat16
        in_maps.append({
            "xT": np.ascontiguousarray(xf[b].T).astype(bf),
            "wqT": np.ascontiguousarray(np.asarray(Wq, np.float32)[hs].T).astype(bf),
            "wkT": np.ascontiguousarray(np.asarray(Wk, np.float32)[hs].T).astype(bf),
            "wvT": np.ascontiguousarray(np.asarray(Wv, np.float32)[hs].T).astype(bf),
            "wpT": np.ascontiguousarray(np.asarray(Wp, np.float32)[:, hs].T).astype(bf),
        })
    return in_maps


def kernel(x, Wq, Wk, Wv, Wp, bp, _trace=False, _tmpdir=None):
    global LAST_RESULTS
    from concourse import bass_utils

    nc = _get_compiled()
    in_maps = make_in_maps(x, Wq, Wk, Wv, Wp)
    kwargs = {}
    if _trace:
        kwargs = {"trace": True, "tmpdir": _tmpdir}
    res = bass_utils.run_bass_kernel_spmd(
        nc, in_maps, core_ids=list(range(N_CORES)), **kwargs)
    LAST_RESULTS = res
    yout = np.zeros((B, S, C), np.float32)
    for c in range(N_CORES):
        yout[c // HPC] += res.results[c]["y"].astype(np.float32)
    yout += np.asarray(bp, np.float32).reshape(1, 1, C)
    return yout.reshape(B, V, L, C)
